# revision 13
# baseline (speedup 1.0000x reference)
"""Trainium2 Bass kernel for nn_Answer_Decoder (B=64, T=24, H=512, E=256, V=32000).

Math notes (vs the reference):
- The attention softmax is over a singleton axis, so aw == 1.0 exactly and
  ctx == concat(question_feat, image_feat) for every step. The attention
  block contributes nothing else to the output and is omitted.
- logits[b,t] = fc(h2[b,t]) where h2 comes from a 3-layer LSTM over
  cur0[t] = concat(emb[answer_seq[:, t]], ctx).

Distribution (8 NeuronCores, no collectives):
- LSTM is replicated on all cores (a 24-step recurrence cannot afford the
  ~5us/call collective floor); the fc projection + logits are tensor-parallel
  over the vocab dim (4000 cols/core). Output is gathered on host.

Per-core layout:
- All matmuls run in bf16 (fp32 moving-operand streams at 1/4 rate on TRN2).
- Gate weights are row-permuted to [i, f, o, g]; gate matmuls are col-group
  packed: partitions 0:64 accumulate gate cols 0:1024 (i|f), partitions
  64:128 accumulate cols 1024:2048 (o|g) concurrently.
- x-projection for all timesteps (+ ctx projection + biases) is precomputed
  into DRAM ("xb") in phase A and injected per-step into PSUM via an
  identity matmul.
- h is transposed each step via PE-transpose (batch-layout -> lhsT layout).
"""

import sys
import types

import numpy as np
import ml_dtypes

import concourse.bass as bass
import concourse.mybir as mybir
import concourse.tile as tile
from concourse import bacc, bass_utils

B, T, H, E, V = 64, 24, 512, 256, 32000
NCORES = 8
VS = V // NCORES  # 4000
G = 4 * H  # 2048
NT = T * B  # 1536
MT = NT // 128  # 12 row tiles of (t, b)

F32 = mybir.dt.float32
BF16 = mybir.dt.bfloat16
I32 = mybir.dt.int32
BF = ml_dtypes.bfloat16

# gate permutation: torch rows [i f g o] -> ours [i f o g]
PERM = np.concatenate(
    [np.arange(0, 512), np.arange(512, 1024), np.arange(1536, 2048), np.arange(1024, 1536)]
)

AF = mybir.ActivationFunctionType
OP = mybir.AluOpType

LAST = None  # last BassKernelResults (for test harness timing)


def _install_trace_shim():
    """Make trace=True / BASS_TRACE survivable in this container."""
    try:
        if "antenv.axon_hooks" not in sys.modules:
            mod = types.ModuleType("antenv.axon_hooks")
            mod._hook = None
            mod.set_axon_ntff_profile_hook = lambda h: setattr(mod, "_hook", h)
            mod.get_axon_ntff_profile_hook = lambda: mod._hook
            sys.modules["antenv.axon_hooks"] = mod
        import antenv.axon_hooks as ah

        if ah.get_axon_ntff_profile_hook() is None:
            try:
                from trn_agent_boot.trn_boot import _ntff_profile_via_ctypes

                ah.set_axon_ntff_profile_hook(
                    _ntff_profile_via_ctypes("/opt/axon/libaxon_pjrt.so")
                )
            except Exception:
                pass
        import concourse.bass_utils as bu

        bu.upload_artifacts = lambda tmpdir: f"local:{tmpdir}"
    except Exception:
        pass


class MMGroup:
    """Collects matmuls targeting one PSUM region; sets start on the first
    and stop on the last when flushed."""

    def __init__(self, nc):
        self.nc = nc
        self.calls = []

    def add(self, out, lhsT, rhs, tile_position=None):
        self.calls.append((out, lhsT, rhs, tile_position))

    def flush(self):
        n = len(self.calls)
        for i, (out, lhsT, rhs, tp) in enumerate(self.calls):
            self.nc.tensor.matmul(
                out,
                lhsT,
                rhs,
                start=(i == 0),
                stop=(i == n - 1),
                tile_position=tp,
            )
        self.calls = []


def build_graph(has_bias, has_fcb):
    nc = bacc.Bacc(None, target_bir_lowering=False)

    # ---- DRAM parameters (already in device layout, bf16 unless noted) ----
    d_emb = nc.declare_dram_parameter("emb", [V, E], F32, isOutput=False)
    d_idx = nc.declare_dram_parameter("idx", [128, MT], I32, isOutput=False)
    d_comb = nc.declare_dram_parameter("combT", [128, 8, 64], BF16, isOutput=False)
    d_wx = nc.declare_dram_parameter("WxT", [128, 2, G], BF16, isOutput=False)
    d_wc = nc.declare_dram_parameter("WcT", [128, 8, G], BF16, isOutput=False)
    d_w0 = nc.declare_dram_parameter("W0T", [128, 4, G], BF16, isOutput=False)
    d_w1 = nc.declare_dram_parameter("W1T", [128, 8, G], BF16, isOutput=False)
    d_w2 = nc.declare_dram_parameter("W2T", [128, 8, G], BF16, isOutput=False)
    d_fcw = nc.declare_dram_parameter("fcWT", [128, 4, VS], BF16, isOutput=False)
    d_fcb = nc.declare_dram_parameter("fcb", [1, VS], BF16, isOutput=False)
    d_id = nc.declare_dram_parameter("ident", [128, 128], BF16, isOutput=False)
    d_idc = nc.declare_dram_parameter("identc", [64, 128], BF16, isOutput=False)
    d_ones = nc.declare_dram_parameter("ones", [1, 128], BF16, isOutput=False)
    d_brow = [
        nc.declare_dram_parameter(f"brow{l}", [1, G], BF16, isOutput=False)
        for l in range(3)
    ]
    d_out = nc.declare_dram_parameter("out", [MT, 128, VS], BF16, isOutput=True)

    # internal DRAM spill for the precomputed x/ctx/bias gate projection
    d_xb = nc.dram_tensor("xbspill", [MT, 128, G], BF16)

    with tile.TileContext(nc) as tc:
        with (
            tc.tile_pool(name="wp", bufs=1) as wp,
            tc.tile_pool(name="state", bufs=1) as sp,
            tc.tile_pool(name="psg", bufs=3, space="PSUM") as psg,
            tc.tile_pool(name="psfc", bufs=2, space="PSUM") as psfc,
        ):
            # ---- small persistents (weights come later, after phase A) ----
            w0 = wp.tile([128, 4, G], BF16)
            ident = wp.tile([128, 128], BF16)
            identc = wp.tile([64, 128], BF16)
            ones = wp.tile([1, 128], BF16)
            brow = [
                wp.tile([1, G], BF16, tag=f"brow{l}", name=f"brow{l}")
                if has_bias[l]
                else None
                for l in range(3)
            ]
            wb_cm = tc.tile_pool(name="wb", bufs=1)
            wb = wb_cm.__enter__()
            w1 = wb.tile([128, 8, G], BF16)
            w2 = wb.tile([128, 8, G], BF16)
            fcw = wb.tile([128, 4, VS], BF16)
            nc.sync.dma_start(w0[:], d_w0[:])
            nc.sync.dma_start(w1[:], d_w1[:])
            nc.sync.dma_start(w2[:], d_w2[:])
            nc.sync.dma_start(fcw[:], d_fcw[:])
            if has_fcb:
                fcb = wb.tile([1, VS], BF16)
                nc.sync.dma_start(fcb[:], d_fcb[:])
            nc.sync.dma_start(ident[:], d_id[:])
            nc.sync.dma_start(identc[:], d_idc[:])
            nc.sync.dma_start(ones[:], d_ones[:])
            for l in range(3):
                if has_bias[l]:
                    nc.sync.dma_start(brow[l][:], d_brow[l][:])
            i64 = ident[0:64, 0:64]
            i64b = ident[64:128, 64:128]  # identity block at base partition 64

            # ---- persistent state (parity-buffered over steps) ----
            hT = [sp.tile([128, 4, 2, 64], BF16, tag=f"h{l}T", name=f"h{l}T") for l in range(3)]
            cst = [sp.tile([64, 2, 512], BF16, tag=f"c{l}", name=f"c{l}") for l in range(3)]
            base_sb = sp.tile([64, G], BF16, tag="base")
            base2 = sp.tile([128, G], BF16, tag="base2")  # base duplicated on both halves

            # =================== phase A ===================
            with tc.tile_pool(name="pa", bufs=1) as pa, tc.tile_pool(
                name="pag", bufs=2
            ) as pag:
                comb = pa.tile([128, 8, 64], BF16)
                wc = pa.tile([128, 8, G], BF16)
                wx = pa.tile([128, 2, G], BF16)
                xt = pa.tile([128, 2, NT], BF16)
                ix = pa.tile([128, MT], I32)
                nc.sync.dma_start(comb[:], d_comb[:])
                nc.sync.dma_start(wc[:], d_wc[:])
                nc.sync.dma_start(wx[:], d_wx[:])
                nc.sync.dma_start(ix[:], d_idx[:])

                # base = ctx @ WcT (+ biases of layer 0): [64, G] in quadrants
                bps = psg.tile([128, 1024], F32, tag="g")
                grp = MMGroup(nc)
                for cg, tp in ((0, (0, 0)), (64, (0, 64))):
                    for c in range(2):
                        dst = bps[cg : cg + 64, c * 512 : (c + 1) * 512]
                        for kt in range(8):
                            grp.add(
                                dst,
                                comb[:, kt, :],
                                wc[:, kt, cg * 16 + c * 512 : cg * 16 + (c + 1) * 512],
                                tile_position=tp,
                            )
                        if has_bias[0]:
                            grp.add(
                                dst,
                                ones[:, 0:64],
                                brow[0][:, cg * 16 + c * 512 : cg * 16 + (c + 1) * 512],
                                tile_position=tp,
                            )
                        grp.flush()
                nc.vector.tensor_copy(out=base_sb[:, 0:1024], in_=bps[0:64, :])
                nc.vector.tensor_copy(out=base_sb[:, 1024:2048], in_=bps[64:128, :])
                nc.vector.tensor_copy(out=base2[0:64, :], in_=base_sb[:])
                nc.vector.tensor_copy(out=base2[64:128, :], in_=base_sb[:])

                # embedding gather -> bf16 -> PE transpose into xt [128,2,NT]
                for m in range(MT):
                    gx = pag.tile([128, E], F32, tag="gx")
                    nc.gpsimd.indirect_dma_start(
                        out=gx[:],
                        out_offset=None,
                        in_=d_emb[:],
                        in_offset=bass.IndirectOffsetOnAxis(ap=ix[:, m : m + 1], axis=0),
                    )
                    gxb = pag.tile([128, E], BF16, tag="gxb")
                    nc.vector.tensor_copy(out=gxb[:], in_=gx[:])
                    xps = psfc.tile([128, 256], BF16, tag="fc")
                    nc.tensor.transpose(xps[:, 0:128], gxb[:, 0:128], ident[:])
                    nc.tensor.transpose(xps[:, 128:256], gxb[:, 128:256], ident[:])
                    nc.vector.tensor_copy(
                        out=xt[:, :, m * 128 : (m + 1) * 128], in_=xps[:]
                    )

                # xproj: xb[m] = X_m @ WxT + stack2(base); spill to DRAM
                for m in range(MT):
                    xst = pag.tile([128, G], BF16, tag="xst")
                    for half in range(2):
                        px = psg.tile([128, 1024], F32, tag="g")
                        grp = MMGroup(nc)
                        for c in range(2):
                            dst = px[:, c * 512 : (c + 1) * 512]
                            off = half * 1024 + c * 512
                            for kt in range(2):
                                grp.add(
                                    dst,
                                    xt[:, kt, m * 128 : (m + 1) * 128],
                                    wx[:, kt, off : off + 512],
                                )
                            grp.flush()
                        nc.vector.tensor_tensor(
                            out=xst[:, half * 1024 : (half + 1) * 1024],
                            in0=px[:],
                            in1=base2[:, half * 1024 : (half + 1) * 1024],
                            op=OP.add,
                        )
                    nc.sync.dma_start(d_xb[m], xst[:])

            # ---- phase B working pools (reuse phase-A space) ----
            pw_cm = tc.tile_pool(name="pw", bufs=2)
            pw = pw_cm.__enter__()
            xbp_cm = tc.tile_pool(name="xbp", bufs=2)
            xbp = xbp_cm.__enter__()
            ostp_cm = tc.tile_pool(name="ost", bufs=2)
            ostp = ostp_cm.__enter__()

            # =================== phase B: recurrence ===================
            def gate_mms(gps, t, layer, xbt):
                """Emit gate matmuls for one layer at step t into gps."""
                p_prev = (t - 1) % 2
                p_cur = t % 2
                if layer == 0:
                    w = w0
                    lhs = [] if t == 0 else [hT[0][:, k, p_prev, :] for k in range(4)]
                elif layer == 1:
                    w = w1
                    lhs = [hT[0][:, k, p_cur, :] for k in range(4)]
                    if t > 0:
                        lhs += [hT[1][:, k, p_prev, :] for k in range(4)]
                else:
                    w = w2
                    lhs = [hT[1][:, k, p_cur, :] for k in range(4)]
                    if t > 0:
                        lhs += [hT[2][:, k, p_prev, :] for k in range(4)]
                # interleave cg0/cg64 so the two col-groups stream
                # concurrently; start/stop tracked per psum region
                for c in range(2):
                    srcs = []
                    if layer == 0:
                        srcs.append((i64, "xb"))
                    for kt, lt in enumerate(lhs):
                        srcs.append((lt, kt))
                    if layer > 0 and has_bias[layer]:
                        srcs.append((ones[:, 0:64], "bias"))
                    n = len(srcs)
                    for i, (lhsT, kind) in enumerate(srcs):
                        for cg, tp in ((0, (0, 0)), (64, (0, 64))):
                            dst = gps[cg : cg + 64, c * 512 : (c + 1) * 512]
                            off = cg * 16 + c * 512
                            if kind == "xb":
                                rhs = xbt[:, off : off + 512]
                            elif kind == "bias":
                                rhs = brow[layer][:, off : off + 512]
                            else:
                                rhs = w[:, kind, off : off + 512]
                            nc.tensor.matmul(
                                dst, lhsT, rhs, start=(i == 0),
                                stop=(i == n - 1), tile_position=tp,
                            )

            def pointwise(gps, t, layer):
                """gates psum -> h (bf16, batch layout) -> hT (transposed)."""
                p_cur = t % 2
                p_prev = (t - 1) % 2
                sio = pw.tile([128, 512], BF16, tag="sio")
                sf = pw.tile([64, 512], BF16, tag="sf")
                tg = pw.tile([64, 512], BF16, tag="tg")
                # i (top) and o (bottom) share a column range: one 128-wide op
                nc.scalar.activation(sio[:], gps[:, 0:512], AF.Sigmoid)
                nc.scalar.activation(sf[:], gps[0:64, 512:1024], AF.Sigmoid)
                nc.scalar.activation(tg[:], gps[64:128, 512:1024], AF.Tanh)
                c_new = cst[layer][:, p_cur, :]
                if t == 0:
                    nc.vector.tensor_tensor(
                        out=c_new, in0=sio[0:64, :], in1=tg[:], op=OP.mult
                    )
                else:
                    t1 = pw.tile([64, 512], BF16, tag="t1")
                    t2 = pw.tile([64, 512], BF16, tag="t2")
                    nc.vector.tensor_tensor(
                        out=t1[:], in0=sf[:], in1=cst[layer][:, p_prev, :],
                        op=OP.mult,
                    )
                    nc.vector.tensor_tensor(
                        out=t2[:], in0=sio[0:64, :], in1=tg[:], op=OP.mult
                    )
                    nc.vector.tensor_tensor(out=c_new, in0=t1[:], in1=t2[:], op=OP.add)
                # tanh(c) and h live on partitions 64:128 so the h-mul reads
                # sigma(o) in place (same base partition)
                htc = pw.tile([128, 512], BF16, tag="htc")
                nc.scalar.activation(htc[64:128, :], c_new, AF.Tanh)
                hsb = pw.tile([128, 512], BF16, tag="hsb")
                nc.vector.tensor_tensor(
                    out=hsb[64:128, :], in0=sio[64:128, :], in1=htc[64:128, :],
                    op=OP.mult,
                )
                trp = psfc.tile([128, 256], BF16, tag="fc")
                for j in range(4):
                    nc.tensor.transpose(
                        trp[:, j * 64 : (j + 1) * 64],
                        hsb[64:128, j * 128 : (j + 1) * 128],
                        i64b,
                    )
                nc.vector.tensor_copy(out=hT[layer][:, :, p_cur, :], in_=trp[:])

            # layer wavefront: tick tau runs L0(tau), L1(tau-1), L2(tau-2)
            # so each tick's pointwise overlaps the next tick's matmuls.
            for tau in range(T + 2):
                units = [(l, tau - l) for l in range(3) if 0 <= tau - l < T]
                gps_map = {}
                xbt = None
                for layer, u in units:
                    if layer == 0:
                        xbt = xbp.tile([64, G], BF16, tag="xbt")
                        nc.sync.dma_start(
                            xbt[:], d_xb[u // 2, (u % 2) * 64 : (u % 2) * 64 + 64, :]
                        )
                    gps = psg.tile([128, 1024], F32, tag="g")
                    gate_mms(gps, u, layer, xbt)
                    gps_map[layer] = gps
                for layer, u in units:
                    pointwise(gps_map[layer], u, layer)

                # fc for the completed (even, odd) step pair
                u2 = tau - 2
                if 0 <= u2 < T and u2 % 2 == 1:
                    s = u2 // 2
                    ost = ostp.tile([128, VS], BF16, tag="ost")
                    for vc in range(8):
                        fps = psfc.tile([128, 500], F32, tag="fc")
                        grp = MMGroup(nc)
                        for kt in range(4):
                            grp.add(
                                fps[:],
                                hT[2][:, kt, :, :],
                                fcw[:, kt, vc * 500 : (vc + 1) * 500],
                            )
                        if has_fcb:
                            grp.add(fps[:], ones[:], fcb[:, vc * 500 : (vc + 1) * 500])
                        grp.flush()
                        dst = ost[:, vc * 500 : (vc + 1) * 500]
                        if vc % 2 == 0:
                            nc.scalar.activation(
                                dst, fps[:], AF.Copy,
                                bias=0.0,
                            )
                        else:
                            nc.vector.tensor_copy(out=dst, in_=fps[:])
                    nc.sync.dma_start(d_out[s], ost[:])

            ostp_cm.__exit__(None, None, None)
            xbp_cm.__exit__(None, None, None)
            pw_cm.__exit__(None, None, None)
            wb_cm.__exit__(None, None, None)

    nc.compile()
    return nc


def _prep(x):
    return np.ascontiguousarray(x)


def _to_bf(x):
    return _prep(np.asarray(x, dtype=np.float32).astype(BF))


def _wt_tiles(wT, n_kt):
    """[K, N] -> [128, n_kt, N] partition-major K tiling."""
    K, N = wT.shape
    assert K == n_kt * 128
    return _prep(wT.reshape(n_kt, 128, N).transpose(1, 0, 2))


def kernel(**inputs):
    _install_trace_shim()

    qf = np.asarray(inputs["question_feat"], np.float32)
    imf = np.asarray(inputs["image_feat"], np.float32)
    seq = np.asarray(inputs["answer_seq"])
    emb = np.asarray(inputs["embedding"], np.float32)
    fc_W = np.asarray(inputs["fc_W"], np.float32)
    fc_b = np.asarray(inputs["fc_b"], np.float32)

    Ws = []
    for l in range(3):
        Ws.append(
            (
                np.asarray(inputs[f"W_ih{l}"], np.float32),
                np.asarray(inputs[f"W_hh{l}"], np.float32),
                np.asarray(inputs[f"b_ih{l}"], np.float32),
                np.asarray(inputs[f"b_hh{l}"], np.float32),
            )
        )

    has_bias = [bool(np.any(Ws[l][2]) or np.any(Ws[l][3])) for l in range(3)]

    # ---- host-side layout prep ----
    comb = np.concatenate([qf, imf], axis=1)  # [B, 2H]
    combT = _wt_tiles(_to_bf(comb.T).astype(BF), 8)  # [128, 8, 64]

    W0p = Ws[0][0][PERM]  # [G, E+2H]
    WxT = _wt_tiles(_to_bf(W0p[:, :E].T), 2)
    WcT = _wt_tiles(_to_bf(W0p[:, E:].T), 8)
    W0T = _wt_tiles(_to_bf(Ws[0][1][PERM].T), 4)
    W1T = _wt_tiles(
        np.concatenate([_to_bf(Ws[1][0][PERM].T), _to_bf(Ws[1][1][PERM].T)], axis=0), 8
    )
    W2T = _wt_tiles(
        np.concatenate([_to_bf(Ws[2][0][PERM].T), _to_bf(Ws[2][1][PERM].T)], axis=0), 8
    )
    brows = [
        _prep((Ws[l][2] + Ws[l][3])[PERM].astype(BF)[None, :]) for l in range(3)
    ]

    idx = _prep(seq.astype(np.int32).T.reshape(MT, 128).T)  # [128, MT]
    ident = _prep(np.eye(128, dtype=np.float32).astype(BF))
    identc = _prep(np.concatenate([np.eye(64), np.eye(64)], axis=1).astype(BF))
    onesm = _prep(np.ones((1, 128), np.float32).astype(BF))

    has_fcb = bool(np.any(fc_b))
    nc = build_graph(has_bias, has_fcb)

    in_maps = []
    for c in range(NCORES):
        fcw_slice = fc_W[c * VS : (c + 1) * VS].T  # [H, VS]
        im = {
            "emb": _prep(emb),
            "idx": idx,
            "combT": combT,
            "WxT": WxT,
            "WcT": WcT,
            "W0T": W0T,
            "W1T": W1T,
            "W2T": W2T,
            "fcWT": _wt_tiles(_to_bf(fcw_slice), 4),
            "fcb": _prep(fc_b[c * VS : (c + 1) * VS].astype(BF)[None, :]),
            "ident": ident,
            "identc": identc,
            "ones": onesm,
            "brow0": brows[0],
            "brow1": brows[1],
            "brow2": brows[2],
        }
        in_maps.append(im)

    res = bass_utils.run_bass_kernel_spmd(
        nc, in_maps, core_ids=list(range(NCORES))
    )
    global LAST
    LAST = res

    # ---- unshard: [MT, 128, VS] rows are (t, b) t-major ----
    parts = []
    for c in range(NCORES):
        o = np.asarray(res.results[c]["out"]).astype(np.float32)
        o = o.reshape(T, B, VS).transpose(1, 0, 2)  # [B, T, VS]
        parts.append(o)
    return np.concatenate(parts, axis=2)  # [B, T, V]


# revision 15
# speedup vs baseline: 1.1484x; 1.1484x over previous
"""Trainium2 Bass kernel for nn_Answer_Decoder (B=64, T=24, H=512, E=256, V=32000).

Math notes (vs the reference):
- The attention softmax is over a singleton axis, so aw == 1.0 exactly and
  ctx == concat(question_feat, image_feat) for every step. The attention
  block contributes nothing else to the output and is omitted.
- logits[b,t] = fc(h2[b,t]) where h2 comes from a 3-layer LSTM over
  cur0[t] = concat(emb[answer_seq[:, t]], ctx).

Distribution (8 NeuronCores, no collectives):
- LSTM is replicated on all cores (a 24-step recurrence cannot afford the
  ~5us/call collective floor); the fc projection + logits are tensor-parallel
  over the vocab dim (4000 cols/core). Output is gathered on host.

Per-core layout:
- All matmuls run in bf16 (fp32 moving-operand streams at 1/4 rate on TRN2).
- Gate weights are row-permuted to [i, f, o, g]; gate matmuls are col-group
  packed: partitions 0:64 accumulate gate cols 0:1024 (i|f), partitions
  64:128 accumulate cols 1024:2048 (o|g) concurrently.
- x-projection for all timesteps (+ ctx projection + biases) is precomputed
  into DRAM ("xb") in phase A and injected per-step into PSUM via an
  identity matmul.
- h is transposed each step via PE-transpose (batch-layout -> lhsT layout).
"""

import sys
import types

import numpy as np
import ml_dtypes

import concourse.bass as bass
import concourse.mybir as mybir
import concourse.tile as tile
from concourse import bacc, bass_utils

B, T, H, E, V = 64, 24, 512, 256, 32000
NCORES = 8
VS = V // NCORES  # 4000
G = 4 * H  # 2048
NT = T * B  # 1536
MT = NT // 128  # 12 row tiles of (t, b)

F32 = mybir.dt.float32
BF16 = mybir.dt.bfloat16
I32 = mybir.dt.int32
BF = ml_dtypes.bfloat16

# gate permutation: torch rows [i f g o] -> ours [i g o f].
# Quadrants after col-group packing of the gate matmul (psum [128, 1024]):
#   [0:64, 0:512]=i  [0:64, 512:1024]=g  [64:128, 0:512]=o  [64:128, 512:1024]=f
# f rows are pre-scaled by 0.5 so sigmoid(f) = 0.5*(1 + tanh(f/2)) shares the
# tanh table with g (one 128-partition ACT op for both).
PERM = np.concatenate(
    [np.arange(0, 512), np.arange(1024, 1536), np.arange(1536, 2048), np.arange(512, 1024)]
)


def _permw(w):
    """Permute gate rows to [i,g,o,f] and pre-scale the f block by 0.5."""
    wp = np.array(w[PERM], dtype=np.float32)
    wp[1536:2048] *= 0.5
    return wp

AF = mybir.ActivationFunctionType
OP = mybir.AluOpType

LAST = None  # last BassKernelResults (for test harness timing)


def _install_trace_shim():
    """Make trace=True / BASS_TRACE survivable in this container."""
    try:
        if "antenv.axon_hooks" not in sys.modules:
            mod = types.ModuleType("antenv.axon_hooks")
            mod._hook = None
            mod.set_axon_ntff_profile_hook = lambda h: setattr(mod, "_hook", h)
            mod.get_axon_ntff_profile_hook = lambda: mod._hook
            sys.modules["antenv.axon_hooks"] = mod
        import antenv.axon_hooks as ah

        if ah.get_axon_ntff_profile_hook() is None:
            try:
                from trn_agent_boot.trn_boot import _ntff_profile_via_ctypes

                ah.set_axon_ntff_profile_hook(
                    _ntff_profile_via_ctypes("/opt/axon/libaxon_pjrt.so")
                )
            except Exception:
                pass
        import concourse.bass_utils as bu

        bu.upload_artifacts = lambda tmpdir: f"local:{tmpdir}"
    except Exception:
        pass


class MMGroup:
    """Collects matmuls targeting one PSUM region; sets start on the first
    and stop on the last when flushed."""

    def __init__(self, nc):
        self.nc = nc
        self.calls = []

    def add(self, out, lhsT, rhs, tile_position=None):
        self.calls.append((out, lhsT, rhs, tile_position))

    def flush(self):
        n = len(self.calls)
        for i, (out, lhsT, rhs, tp) in enumerate(self.calls):
            self.nc.tensor.matmul(
                out,
                lhsT,
                rhs,
                start=(i == 0),
                stop=(i == n - 1),
                tile_position=tp,
            )
        self.calls = []


def build_graph(has_bias, has_fcb):
    nc = bacc.Bacc(None, target_bir_lowering=False)

    # ---- DRAM parameters (already in device layout, bf16 unless noted) ----
    d_emb = nc.declare_dram_parameter("emb", [V, E], F32, isOutput=False)
    d_idx = nc.declare_dram_parameter("idx", [128, MT], I32, isOutput=False)
    d_comb = nc.declare_dram_parameter("combT", [128, 8, 64], BF16, isOutput=False)
    d_wx = nc.declare_dram_parameter("WxT", [128, 2, G], BF16, isOutput=False)
    d_wc = nc.declare_dram_parameter("WcT", [128, 8, G], BF16, isOutput=False)
    d_w0 = nc.declare_dram_parameter("W0T", [128, 4, G], BF16, isOutput=False)
    d_w1 = nc.declare_dram_parameter("W1T", [128, 8, G], BF16, isOutput=False)
    d_w2 = nc.declare_dram_parameter("W2T", [128, 8, G], BF16, isOutput=False)
    d_fcw = nc.declare_dram_parameter("fcWT", [128, 4, VS], BF16, isOutput=False)
    d_fcb = nc.declare_dram_parameter("fcb", [1, VS], BF16, isOutput=False)
    d_id = nc.declare_dram_parameter("ident", [128, 128], BF16, isOutput=False)
    d_idc = nc.declare_dram_parameter("identc", [64, 128], BF16, isOutput=False)
    d_ones = nc.declare_dram_parameter("ones", [1, 128], BF16, isOutput=False)
    d_brow = [
        nc.declare_dram_parameter(f"brow{l}", [1, G], BF16, isOutput=False)
        for l in range(3)
    ]
    d_out = nc.declare_dram_parameter("out", [MT, 128, VS], BF16, isOutput=True)

    # internal DRAM spill for the precomputed x/ctx/bias gate projection
    d_xb = nc.dram_tensor("xbspill", [MT, 128, G], BF16)

    with tile.TileContext(nc) as tc:
        with (
            tc.tile_pool(name="wp", bufs=1) as wp,
            tc.tile_pool(name="state", bufs=1) as sp,
            tc.tile_pool(name="psg", bufs=3, space="PSUM") as psg,
            tc.tile_pool(name="psfc", bufs=2, space="PSUM") as psfc,
        ):
            # ---- small persistents (weights come later, after phase A) ----
            w0 = wp.tile([128, 4, G], BF16)
            ident = wp.tile([128, 128], BF16)
            identc = wp.tile([64, 128], BF16)
            ones = wp.tile([1, 128], BF16)
            brow = [
                wp.tile([1, G], BF16, tag=f"brow{l}", name=f"brow{l}")
                if has_bias[l]
                else None
                for l in range(3)
            ]
            wb_cm = tc.tile_pool(name="wb", bufs=1)
            wb = wb_cm.__enter__()
            w1 = wb.tile([128, 8, G], BF16)
            w2 = wb.tile([128, 8, G], BF16)
            fcw = wb.tile([128, 4, VS], BF16)
            nc.sync.dma_start(w0[:], d_w0[:])
            nc.sync.dma_start(w1[:], d_w1[:])
            nc.sync.dma_start(w2[:], d_w2[:])
            nc.sync.dma_start(fcw[:], d_fcw[:])
            if has_fcb:
                fcb = wb.tile([1, VS], BF16)
                nc.sync.dma_start(fcb[:], d_fcb[:])
            nc.sync.dma_start(ident[:], d_id[:])
            nc.sync.dma_start(identc[:], d_idc[:])
            nc.sync.dma_start(ones[:], d_ones[:])
            for l in range(3):
                if has_bias[l]:
                    nc.sync.dma_start(brow[l][:], d_brow[l][:])
            i64 = ident[0:64, 0:64]
            i64b = ident[64:128, 64:128]  # identity block at base partition 64

            # ---- persistent state (parity-buffered over steps) ----
            hT = [sp.tile([128, 4, 2, 64], BF16, tag=f"h{l}T", name=f"h{l}T") for l in range(3)]
            cst = [[sp.tile([128, 512], BF16, tag=f"c{l}p{p}", name=f"c{l}p{p}") for p in range(2)] for l in range(3)]
            base_sb = sp.tile([64, G], BF16, tag="base")
            base2 = sp.tile([128, G], BF16, tag="base2")  # base duplicated on both halves

            # =================== phase A ===================
            with tc.tile_pool(name="pa", bufs=1) as pa, tc.tile_pool(
                name="pag", bufs=2
            ) as pag:
                comb = pa.tile([128, 8, 64], BF16)
                wc = pa.tile([128, 8, G], BF16)
                wx = pa.tile([128, 2, G], BF16)
                xt = pa.tile([128, 2, NT], BF16)
                ix = pa.tile([128, MT], I32)
                nc.sync.dma_start(comb[:], d_comb[:])
                nc.sync.dma_start(wc[:], d_wc[:])
                nc.sync.dma_start(wx[:], d_wx[:])
                nc.sync.dma_start(ix[:], d_idx[:])

                # base = ctx @ WcT (+ biases of layer 0): [64, G] in quadrants
                bps = psg.tile([128, 1024], F32, tag="g")
                grp = MMGroup(nc)
                for cg, tp in ((0, (0, 0)), (64, (0, 64))):
                    for c in range(2):
                        dst = bps[cg : cg + 64, c * 512 : (c + 1) * 512]
                        for kt in range(8):
                            grp.add(
                                dst,
                                comb[:, kt, :],
                                wc[:, kt, cg * 16 + c * 512 : cg * 16 + (c + 1) * 512],
                                tile_position=tp,
                            )
                        if has_bias[0]:
                            grp.add(
                                dst,
                                ones[:, 0:64],
                                brow[0][:, cg * 16 + c * 512 : cg * 16 + (c + 1) * 512],
                                tile_position=tp,
                            )
                        grp.flush()
                nc.vector.tensor_copy(out=base_sb[:, 0:1024], in_=bps[0:64, :])
                nc.vector.tensor_copy(out=base_sb[:, 1024:2048], in_=bps[64:128, :])
                nc.vector.tensor_copy(out=base2[0:64, :], in_=base_sb[:])
                nc.vector.tensor_copy(out=base2[64:128, :], in_=base_sb[:])

                # embedding gather -> bf16 -> PE transpose into xt [128,2,NT]
                for m in range(MT):
                    gx = pag.tile([128, E], F32, tag="gx")
                    nc.gpsimd.indirect_dma_start(
                        out=gx[:],
                        out_offset=None,
                        in_=d_emb[:],
                        in_offset=bass.IndirectOffsetOnAxis(ap=ix[:, m : m + 1], axis=0),
                    )
                    gxb = pag.tile([128, E], BF16, tag="gxb")
                    nc.vector.tensor_copy(out=gxb[:], in_=gx[:])
                    xps = psfc.tile([128, 256], BF16, tag="fc")
                    nc.tensor.transpose(xps[:, 0:128], gxb[:, 0:128], ident[:])
                    nc.tensor.transpose(xps[:, 128:256], gxb[:, 128:256], ident[:])
                    nc.vector.tensor_copy(
                        out=xt[:, :, m * 128 : (m + 1) * 128], in_=xps[:]
                    )

                # xproj: xb[m] = X_m @ WxT + stack2(base); spill to DRAM
                for m in range(MT):
                    xst = pag.tile([128, G], BF16, tag="xst")
                    for half in range(2):
                        px = psg.tile([128, 1024], F32, tag="g")
                        grp = MMGroup(nc)
                        for c in range(2):
                            dst = px[:, c * 512 : (c + 1) * 512]
                            off = half * 1024 + c * 512
                            for kt in range(2):
                                grp.add(
                                    dst,
                                    xt[:, kt, m * 128 : (m + 1) * 128],
                                    wx[:, kt, off : off + 512],
                                )
                            grp.flush()
                        nc.vector.tensor_tensor(
                            out=xst[:, half * 1024 : (half + 1) * 1024],
                            in0=px[:],
                            in1=base2[:, half * 1024 : (half + 1) * 1024],
                            op=OP.add,
                        )
                    nc.sync.dma_start(d_xb[m], xst[:])

            # ---- phase B working pools (reuse phase-A space) ----
            pw_cm = tc.tile_pool(name="pw", bufs=2)
            pw = pw_cm.__enter__()
            xbp_cm = tc.tile_pool(name="xbp", bufs=2)
            xbp = xbp_cm.__enter__()
            ostp_cm = tc.tile_pool(name="ost", bufs=2)
            ostp = ostp_cm.__enter__()

            # =================== phase B: recurrence ===================
            def gate_mms(gps, t, layer, xbt):
                """Emit gate matmuls for one layer at step t into gps."""
                p_prev = (t - 1) % 2
                p_cur = t % 2
                if layer == 0:
                    w = w0
                    lhs = [] if t == 0 else [hT[0][:, k, p_prev, :] for k in range(4)]
                elif layer == 1:
                    w = w1
                    lhs = [hT[0][:, k, p_cur, :] for k in range(4)]
                    if t > 0:
                        lhs += [hT[1][:, k, p_prev, :] for k in range(4)]
                else:
                    w = w2
                    lhs = [hT[1][:, k, p_cur, :] for k in range(4)]
                    if t > 0:
                        lhs += [hT[2][:, k, p_prev, :] for k in range(4)]
                # interleave cg0/cg64 so the two col-groups stream
                # concurrently; start/stop tracked per psum region
                for c in range(2):
                    srcs = []
                    if layer == 0:
                        srcs.append((i64, "xb"))
                    for kt, lt in enumerate(lhs):
                        srcs.append((lt, kt))
                    if layer > 0 and has_bias[layer]:
                        srcs.append((ones[:, 0:64], "bias"))
                    n = len(srcs)
                    for i, (lhsT, kind) in enumerate(srcs):
                        for cg, tp in ((0, (0, 0)), (64, (0, 64))):
                            dst = gps[cg : cg + 64, c * 512 : (c + 1) * 512]
                            off = cg * 16 + c * 512
                            if kind == "xb":
                                rhs = xbt[:, off : off + 512]
                            elif kind == "bias":
                                rhs = brow[layer][:, off : off + 512]
                            else:
                                rhs = w[:, kind, off : off + 512]
                            nc.tensor.matmul(
                                dst, lhsT, rhs, start=(i == 0),
                                stop=(i == n - 1), tile_position=tp,
                            )

            def pointwise(gps, t, layer):
                """gates psum -> h (bf16, batch layout) -> hT (transposed)."""
                sio = pw.tile([128, 512], BF16, tag="sio")
                tgf = pw.tile([128, 512], BF16, tag="tgf")
                # quadrants: (i|o) share cols 0:512, (g|f) share cols 512:1024
                # -> two 128-partition ACT ops cover all four gates
                nc.scalar.activation(sio[:], gps[:, 0:512], AF.Sigmoid)
                nc.scalar.activation(tgf[:], gps[:, 512:1024], AF.Tanh)
                c_new = cst[layer][t % 2][64:128, :]
                if t == 0:
                    # c = sigma(i)*tanh(g); write at base 64 for later ops
                    nc.vector.tensor_tensor(
                        out=c_new, in0=sio[0:64, :], in1=tgf[0:64, :], op=OP.mult
                    )
                else:
                    a64 = pw.tile([128, 512], BF16, tag="a64")
                    ctmp = pw.tile([128, 512], BF16, tag="ctmp")
                    nc.vector.tensor_tensor(
                        out=a64[64:128, :], in0=sio[0:64, :], in1=tgf[0:64, :],
                        op=OP.mult,
                    )
                    # 2*sigma(f)*c_prev = (tanh(f/2)+1)*c_prev
                    nc.vector.scalar_tensor_tensor(
                        out=ctmp[64:128, :], in0=tgf[64:128, :], scalar=1.0,
                        in1=cst[layer][(t - 1) % 2][64:128, :],
                        op0=OP.add, op1=OP.mult,
                    )
                    nc.vector.scalar_tensor_tensor(
                        out=c_new, in0=ctmp[64:128, :], scalar=0.5,
                        in1=a64[64:128, :], op0=OP.mult, op1=OP.add,
                    )
                htc = pw.tile([128, 512], BF16, tag="htc")
                nc.scalar.activation(htc[64:128, :], c_new, AF.Tanh)
                hsb = pw.tile([128, 512], BF16, tag="hsb")
                nc.vector.tensor_tensor(
                    out=hsb[64:128, :], in0=sio[64:128, :], in1=htc[64:128, :],
                    op=OP.mult,
                )
                trp = psfc.tile([128, 256], BF16, tag="fc")
                for j in range(4):
                    nc.tensor.transpose(
                        trp[:, j * 64 : (j + 1) * 64],
                        hsb[64:128, j * 128 : (j + 1) * 128],
                        i64b,
                    )
                nc.vector.tensor_copy(out=hT[layer][:, :, t % 2, :], in_=trp[:])

            # layer wavefront: tick tau runs L0(tau), L1(tau-1), L2(tau-2)
            # so each tick's pointwise overlaps the next tick's matmuls.
            for tau in range(T + 2):
                units = [(l, tau - l) for l in range(3) if 0 <= tau - l < T]
                gps_map = {}
                xbt = None
                for layer, u in units:
                    if layer == 0:
                        xbt = xbp.tile([64, G], BF16, tag="xbt")
                        nc.sync.dma_start(
                            xbt[:], d_xb[u // 2, (u % 2) * 64 : (u % 2) * 64 + 64, :]
                        )
                    gps = psg.tile([128, 1024], F32, tag="g")
                    gate_mms(gps, u, layer, xbt)
                    gps_map[layer] = gps
                for layer, u in units:
                    pointwise(gps_map[layer], u, layer)

                # fc for the completed (even, odd) step pair
                u2 = tau - 2
                if 0 <= u2 < T and u2 % 2 == 1:
                    s = u2 // 2
                    ost = ostp.tile([128, VS], BF16, tag="ost")
                    for vc in range(8):
                        fps = psfc.tile([128, 500], F32, tag="fc")
                        grp = MMGroup(nc)
                        for kt in range(4):
                            grp.add(
                                fps[:],
                                hT[2][:, kt, :, :],
                                fcw[:, kt, vc * 500 : (vc + 1) * 500],
                            )
                        if has_fcb:
                            grp.add(fps[:], ones[:], fcb[:, vc * 500 : (vc + 1) * 500])
                        grp.flush()
                        dst = ost[:, vc * 500 : (vc + 1) * 500]
                        if vc % 2 == 0:
                            nc.scalar.activation(
                                dst, fps[:], AF.Copy,
                                bias=0.0,
                            )
                        else:
                            nc.vector.tensor_copy(out=dst, in_=fps[:])
                    nc.sync.dma_start(d_out[s], ost[:])

            ostp_cm.__exit__(None, None, None)
            xbp_cm.__exit__(None, None, None)
            pw_cm.__exit__(None, None, None)
            wb_cm.__exit__(None, None, None)

    nc.compile()
    return nc


def _prep(x):
    return np.ascontiguousarray(x)


def _to_bf(x):
    return _prep(np.asarray(x, dtype=np.float32).astype(BF))


def _wt_tiles(wT, n_kt):
    """[K, N] -> [128, n_kt, N] partition-major K tiling."""
    K, N = wT.shape
    assert K == n_kt * 128
    return _prep(wT.reshape(n_kt, 128, N).transpose(1, 0, 2))


def kernel(**inputs):
    _install_trace_shim()

    qf = np.asarray(inputs["question_feat"], np.float32)
    imf = np.asarray(inputs["image_feat"], np.float32)
    seq = np.asarray(inputs["answer_seq"])
    emb = np.asarray(inputs["embedding"], np.float32)
    fc_W = np.asarray(inputs["fc_W"], np.float32)
    fc_b = np.asarray(inputs["fc_b"], np.float32)

    Ws = []
    for l in range(3):
        Ws.append(
            (
                np.asarray(inputs[f"W_ih{l}"], np.float32),
                np.asarray(inputs[f"W_hh{l}"], np.float32),
                np.asarray(inputs[f"b_ih{l}"], np.float32),
                np.asarray(inputs[f"b_hh{l}"], np.float32),
            )
        )

    has_bias = [bool(np.any(Ws[l][2]) or np.any(Ws[l][3])) for l in range(3)]

    # ---- host-side layout prep ----
    comb = np.concatenate([qf, imf], axis=1)  # [B, 2H]
    combT = _wt_tiles(_to_bf(comb.T).astype(BF), 8)  # [128, 8, 64]

    W0p = _permw(Ws[0][0])  # [G, E+2H]
    WxT = _wt_tiles(_to_bf(W0p[:, :E].T), 2)
    WcT = _wt_tiles(_to_bf(W0p[:, E:].T), 8)
    W0T = _wt_tiles(_to_bf(_permw(Ws[0][1]).T), 4)
    W1T = _wt_tiles(
        np.concatenate([_to_bf(_permw(Ws[1][0]).T), _to_bf(_permw(Ws[1][1]).T)], axis=0), 8
    )
    W2T = _wt_tiles(
        np.concatenate([_to_bf(_permw(Ws[2][0]).T), _to_bf(_permw(Ws[2][1]).T)], axis=0), 8
    )
    brows = [
        _prep(_permw((Ws[l][2] + Ws[l][3])[:, None])[:, 0].astype(BF)[None, :]) for l in range(3)
    ]

    idx = _prep(seq.astype(np.int32).T.reshape(MT, 128).T)  # [128, MT]
    ident = _prep(np.eye(128, dtype=np.float32).astype(BF))
    identc = _prep(np.concatenate([np.eye(64), np.eye(64)], axis=1).astype(BF))
    onesm = _prep(np.ones((1, 128), np.float32).astype(BF))

    has_fcb = bool(np.any(fc_b))
    nc = build_graph(has_bias, has_fcb)

    in_maps = []
    for c in range(NCORES):
        fcw_slice = fc_W[c * VS : (c + 1) * VS].T  # [H, VS]
        im = {
            "emb": _prep(emb),
            "idx": idx,
            "combT": combT,
            "WxT": WxT,
            "WcT": WcT,
            "W0T": W0T,
            "W1T": W1T,
            "W2T": W2T,
            "fcWT": _wt_tiles(_to_bf(fcw_slice), 4),
            "fcb": _prep(fc_b[c * VS : (c + 1) * VS].astype(BF)[None, :]),
            "ident": ident,
            "identc": identc,
            "ones": onesm,
            "brow0": brows[0],
            "brow1": brows[1],
            "brow2": brows[2],
        }
        in_maps.append(im)

    res = bass_utils.run_bass_kernel_spmd(
        nc, in_maps, core_ids=list(range(NCORES))
    )
    global LAST
    LAST = res

    # ---- unshard: [MT, 128, VS] rows are (t, b) t-major ----
    parts = []
    for c in range(NCORES):
        o = np.asarray(res.results[c]["out"]).astype(np.float32)
        o = o.reshape(T, B, VS).transpose(1, 0, 2)  # [B, T, VS]
        parts.append(o)
    return np.concatenate(parts, axis=2)  # [B, T, V]
